# revision 27
# baseline (speedup 1.0000x reference)
"""DKNN (retrieval_knn) Trainium2 Bass kernel — v5.

Full inputs: query [128, 256] f32, neighbors [1024, 256] f32.
Output: [128, 1024] f32 = sum of first K=10 NeuralSort P_hat rows.

Sharding: query batch split 16/core across 8 cores; neighbors replicated.

Math (per core, Q=16, M=1024, D=256, K=10, tau=1):
  u[q,m]   = 2*query[q]@n[m] - |n[m]|^2   (score + per-q const, cancels in
                                           softmax); then u -= max_m u[q,m]
  acc[q,j] = sum_m relu(u[q,m] - u[q,j])
  r = 2*acc + M*u - sum(u)
  logits[q,i,j] = u_j*(M-1-2i) - r_j
               == -(1+2i)*u_j - 2*acc_j + row_const   (M*u folds into the
                  coef exactly since M = 2^10; row consts cancel in softmax)
  out[q,j] = sum_i softmax_j(logits[q,i,:])

Numerics: acc partial sums are tiny exactly at the top-ranked j where the
softmax mass lives, so fp32 accumulation / fp32r PE-reduction noise lands
only on logits thousands below the row max. The -(1+2i) coefficient also
kills the ~1000x amplification of u's fp32 matmul noise (this was the
previous version's dominant error).

Engine split of the O(Q*M^2) work, per HW-measured op costs (gpsimd
supports no accumulating/2-tensor ops and its plain tensor_scalar ucode
is ~15us per [128,1024] op, so Pool only does the q0 broadcast; ACT
SBUF-source ops pay the 2.3x cayman errata):
  M-queries (most): 8 ops [p=m-chunk, free=j] on DVE:
      tensor_scalar min(u_j - u_m, 0) -> fp32r scratch (~594ns, 2x_2p);
      PE one-hot(-1) fp32r matmuls accumulate partial acc rows straight
      into r_ps PSUM (~480ns/op, PE is otherwise idle).
  J-queries (a few): 8 ops [p=j-chunk, free=m] on ACT (fused
      activation(Relu, bias=-u_j, accum_out) -> acc columns, ~2.1us) and
      a few on DVE (scalar_tensor_tensor + accum, ~1.2us); columns are
      PE-transposed (accumulating) into the same r_ps rows.

ubc (u[q,:] broadcast to 128 partitions) serves BOTH layouts (free dim
is m or j respectively). Engine SBUF APs must start at partition
0/32/64/96, so only q0 can use gpsimd partition_broadcast; the rest are
SP-queue DMA broadcast reads of a DRAM u copy (0-partition-stride AP),
delivered J-queries-first so ACT starts early.

Epilogue on PE: l[p,j] = coef'(p)*u[q(p),j] - 2*acc[q(p),j] built in
PSUM via one-hot replicate matmuls (u-terms early; -2*acc terms per
quarter as r_sb lands), then row-max + exp(+accum) on ACT, 1/z folded
into the final K-row-sum fp32r matmul.
"""

import os
from contextlib import ExitStack

import numpy as np

import concourse.bacc as bacc
import concourse.bass as bass
import concourse.tile as tile
import concourse.mybir as mybir
from concourse.bass_utils import run_bass_kernel_spmd

F32 = mybir.dt.float32
F32R = mybir.dt.float32r
I32 = mybir.dt.int32
AX = mybir.AxisListType
OP = mybir.AluOpType
AF = mybir.ActivationFunctionType

QF = 128          # full query batch
Q = 16            # queries per core
M = 1024          # neighbors
D = 256           # dim
K = 10            # top-k / NeuralSort rows
NCORES = 8
NCH = 8           # 128-row chunks per query (j-chunks or m-chunks)

# --- tuning knobs ---------------------------------------------------------
NJQ = int(os.environ.get("DKNN_NJQ", "4"))       # queries done J-side
# per-op engine cost estimates (ns) for the build-time list scheduler
COST_A = float(os.environ.get("DKNN_COST_A", "2110"))    # ACT J fused
COST_DJ = float(os.environ.get("DKNN_COST_DJ", "1200"))  # DVE J stt
COST_DM = float(os.environ.get("DKNN_COST_DM", "620"))   # DVE M pass
COST_PB = 1300.0          # pool partition_broadcast (q0)
COST_DB = 1650.0          # SP dma broadcast (queue-serial)


def j_queries():
    """J-side (ACT) queries: early ubc-delivery slots, interleaved with
    M-queries so DVE never starves."""
    return set(range(1, 2 * NJQ, 2))


def bcast_order():
    """SP DMA delivery order for ubc[1..15]: alternate J/M early."""
    jq = sorted(j_queries())
    mqs = [q for q in range(1, Q) if q not in jq]
    order = []
    i = j = 0
    while i < len(jq) or j < len(mqs):
        if i < len(jq):
            order.append(jq[i]); i += 1
        if j < len(mqs):
            order.append(mqs[j]); j += 1
    return order


def schedule_main_ops():
    """Greedy earliest-finish schedule.

    Returns ops as (kind, q, c, engine): kind 'J' (c = j-chunk) or 'M'
    (c = m-chunk). Engines: A (J-fused), D (M-pass or J-stt).
    Estimates guide balance only; correctness is schedule-independent.
    """
    jq = j_queries()
    ready = {0: COST_PB}
    t = 1650.0
    for q in bcast_order():
        t += COST_DB
        ready[q] = t

    jops = [(c, q) for c in range(NCH) for q in sorted(jq)]
    mops = [(mc, q) for q in range(Q) if q not in jq for mc in range(NCH)]
    avail = {"A": 0.0, "D": 0.0}
    sched = []
    while jops or mops:
        e = min("AD", key=lambda x: avail[x])
        if e == "A":
            if not jops:
                avail["A"] = float("inf")
                continue
            pool, cost, kind = jops, COST_A, "J"
        else:
            if mops:
                pool, cost, kind = mops, COST_DM, "M"
            else:
                pool, cost, kind = jops, COST_DJ, "J"
        op = min(pool, key=lambda cq: (max(ready[cq[1]], avail[e]), cq[0]))
        pool.remove(op)
        avail[e] = max(avail[e], ready[op[1]]) + cost
        sched.append((kind, op[1], op[0], e))
    return sched


def build_kernel():
    nc = bacc.Bacc(
        "TRN2",
        target_bir_lowering=False,
        debug=False,
        enable_asserts=False,
        num_devices=NCORES,
    )

    q_dram = nc.dram_tensor("query", [Q, D], F32, kind="ExternalInput")
    n_dram = nc.dram_tensor("neighbors", [M, D], F32, kind="ExternalInput")
    out_dram = nc.dram_tensor("out", [Q, M], F32, kind="ExternalOutput")
    dump = os.environ.get("DKNN_DEBUG_DUMP", "0") == "1"
    u_dram = nc.dram_tensor("u_scratch", [Q, M], F32,
                            kind="ExternalOutput" if dump else "Internal")
    r_dram = nc.dram_tensor("r_scratch", [Q, M], F32, kind="ExternalOutput") \
        if dump else None

    loop_n = int(os.environ.get("DKNN_LOOP", "1"))
    with tile.TileContext(nc) as tc:
        with ExitStack() as ctx:
            if loop_n > 1:
                with tc.For_i(0, loop_n, 1):
                    kernel_body(ctx, tc, q_dram, n_dram, out_dram, u_dram,
                                r_dram)
            else:
                kernel_body(ctx, tc, q_dram, n_dram, out_dram, u_dram,
                            r_dram)

    nc.compile()
    return nc


def kernel_body(ctx, tc, q_dram, n_dram, out_dram, u_dram, r_dram=None):
    nc = tc.nc

    const = ctx.enter_context(tc.tile_pool(name="const", bufs=1))
    big = ctx.enter_context(tc.tile_pool(name="big", bufs=1))
    epil = ctx.enter_context(tc.tile_pool(name="epil", bufs=1))
    ps_a = ctx.enter_context(tc.tile_pool(name="ps_a", bufs=2, space="PSUM"))
    ps_sc = ctx.enter_context(tc.tile_pool(name="ps_sc", bufs=1, space="PSUM"))
    ps_l = ctx.enter_context(tc.tile_pool(name="ps_l", bufs=1, space="PSUM"))
    ps_l2 = ctx.enter_context(tc.tile_pool(name="ps_l2", bufs=1, space="PSUM"))

    sched = schedule_main_ops()
    have_m = any(k == "M" for k, _, _, _ in sched)

    # ---------------- constants (no data deps) -----------------------------
    id128 = const.tile([128, 128], F32)
    idi = const.tile([128, 128], I32)
    nc.gpsimd.iota(idi[:], pattern=[[-1, 128]], base=0, channel_multiplier=1)
    nc.vector.tensor_scalar(idi[:], idi[:], 0, None, op0=OP.is_equal)
    nc.vector.tensor_copy(id128[:], idi[:])

    negones = const.tile([128, Q], F32)
    nc.vector.memset(negones[:], -1.0)
    zeros = const.tile([128, M], F32)
    nc.vector.memset(zeros[:], 0.0)

    # one-hot stack for M-side reduction: ohq[k, q, i] = -(i == q), fp32r
    ohi = const.tile([128, Q, Q], I32)
    nc.gpsimd.iota(ohi[:], pattern=[[-1, Q], [1, Q]], base=0,
                   channel_multiplier=0)
    nc.vector.tensor_scalar(ohi[:], ohi[:], 0, None, op0=OP.is_equal)
    ohq = const.tile([128, Q, Q], F32R)
    nc.vector.tensor_copy(ohq[:], ohi[:])
    nc.vector.tensor_scalar(ohq[:], ohq[:], -1.0, None, op0=OP.mult)

    # esel[k, p] one-hot replicators with coef'/-2 folded in
    def repsel(nparts, rep, scale_col=None, scale=None, tag=""):
        vi = epil.tile([Q, nparts], I32, tag=f"vi{tag}")
        nc.gpsimd.iota(vi[:], pattern=[[1, nparts]], base=0,
                       channel_multiplier=-rep)
        nc.vector.tensor_scalar(vi[:], vi[:], rep.bit_length() - 1, None,
                                op0=OP.arith_shift_right)
        nc.vector.tensor_scalar(vi[:], vi[:], 0, None, op0=OP.is_equal)
        sf = epil.tile([Q, nparts], F32, tag=f"sf{tag}")
        nc.vector.tensor_copy(sf[:], vi[:])
        if scale_col is not None:
            nc.vector.tensor_tensor(sf[:], sf[:], scale_col[:], op=OP.mult)
        if scale is not None:
            nc.vector.tensor_scalar(sf[:], sf[:], scale, None, op0=OP.mult)
        return sf

    # coef'(p) = -(1 + 2*i(p)): i = p%8 on 128 parts, 8 + p%2 on 32 parts
    def coef_row(nparts, mask, base_val, tag):
        it = epil.tile([Q, nparts], I32, tag=f"it{tag}")
        nc.gpsimd.iota(it[:], pattern=[[1, nparts]], base=0,
                       channel_multiplier=0)
        nc.vector.tensor_scalar(it[:], it[:], mask, None, op0=OP.bitwise_and)
        ft = epil.tile([Q, nparts], F32, tag=f"ft{tag}")
        nc.vector.tensor_copy(ft[:], it[:])
        nc.vector.tensor_scalar(ft[:], ft[:], -2.0, float(base_val),
                                op0=OP.mult, op1=OP.add)
        return ft

    coefr1 = coef_row(128, 7, -1, "a")
    coefr2 = coef_row(32, 1, -17, "b")
    esel_coef = repsel(128, 8, scale_col=coefr1, tag="ec")
    esel2_coef = repsel(32, 2, scale_col=coefr2, tag="ec2")
    esel_neg = repsel(128, 8, scale=-2.0, tag="en")
    esel2_neg = repsel(32, 2, scale=-2.0, tag="en2")

    # ---------------- load inputs ------------------------------------------
    n_sb = big.tile([128, NCH, D], F32)
    n_re = n_dram.ap().rearrange("(mc p) d -> p mc d", p=128)
    for mc in range(NCH):
        eng = (nc.sync, nc.scalar)[mc % 2]
        eng.dma_start(n_sb[:, mc, :], n_re[:, mc, :])

    q_sb = const.tile([Q, D], F32)
    nc.sync.dma_start(q_sb[:], q_dram[:, :])

    # nT[p, dc, m] = neighbors[m, 128*dc + p] via 16 PE transposes
    nT = big.tile([128, 2, M], F32)
    for dc in range(2):
        for hh in range(2):
            psT = ps_a.tile([128, 512], F32, tag="a")
            for mi in range(4):
                mc = hh * 4 + mi
                nc.tensor.transpose(
                    psT[:, mi * 128:(mi + 1) * 128],
                    n_sb[:, mc, dc * 128:(dc + 1) * 128],
                    id128[:],
                )
            eng = nc.scalar if (dc + hh) % 2 == 0 else nc.vector
            if eng is nc.scalar:
                eng.activation(nT[:, dc, hh * 512:(hh + 1) * 512], psT[:],
                               AF.Copy)
            else:
                eng.tensor_copy(nT[:, dc, hh * 512:(hh + 1) * 512], psT[:])

    # q2T[p, dc, q] = 2 * query[q, 128*dc + p]
    q2T = const.tile([128, 2, Q], F32)
    psQ = ps_a.tile([128, 2 * Q], F32, tag="a")
    for dc in range(2):
        nc.tensor.transpose(
            psQ[:, dc * Q:(dc + 1) * Q],
            q_sb[:, dc * 128:(dc + 1) * 128],
            id128[:Q, :Q],
        )
    nc.vector.tensor_scalar(
        q2T[:].rearrange("p a b -> p (a b)"), psQ[:], 2.0, None, op0=OP.mult)

    # nsq = nT*nT (the constant D-offset of u is m-independent and cancels
    # via max-centering; PSUM partial noise is tolerable now that the
    # logit coefficient is -(1+2i) rather than ~1000)
    nsq = big.tile([128, 2, M], F32)
    nc.vector.tensor_tensor(nsq[:], nT[:], nT[:], op=OP.mult)

    # u[q, m] = sum_d 2*q[q,d]*n[m,d] - sum_d n[m,d]^2
    scores_ps = ps_sc.tile([Q, M], F32, tag="sc")
    for h in range(2):  # 512-column halves (fp32 moving-operand limit)
        cols = slice(h * 512, (h + 1) * 512)
        nc.tensor.matmul(scores_ps[:, cols], q2T[:, 0, :], nT[:, 0, cols],
                         start=True, stop=False)
        nc.tensor.matmul(scores_ps[:, cols], q2T[:, 1, :], nT[:, 1, cols],
                         start=False, stop=False)
        nc.tensor.matmul(scores_ps[:, cols], negones[:], nsq[:, 0, cols],
                         start=False, stop=False)
        nc.tensor.matmul(scores_ps[:, cols], negones[:], nsq[:, 1, cols],
                         start=False, stop=True)

    # u to SBUF, max-centered per query (folded into the PSUM->SBUF copy)
    umax = const.tile([Q, 1], F32)
    umaxn = const.tile([Q, 1], F32)
    nc.vector.tensor_reduce(umax[:], scores_ps[:], axis=AX.X, op=OP.max)
    nc.vector.tensor_scalar(umaxn[:], umax[:], -1.0, None, op0=OP.mult)
    u_sb = const.tile([Q, M], F32)
    nc.scalar.activation(u_sb[:], scores_ps[:], AF.Identity, bias=umaxn[:],
                         scale=1.0)
    nc.sync.dma_start(u_dram[:, :], u_sb[:])

    # ucol[p, c, q] = u[q, 128*c + p], plus negated copy (ACT relu bias)
    ucol = const.tile([128, NCH, Q], F32)
    ucol_ps = ps_a.tile([128, NCH * Q], F32, tag="a")
    for c in range(NCH):
        nc.tensor.transpose(ucol_ps[:, c * Q:(c + 1) * Q],
                            u_sb[:, c * 128:(c + 1) * 128], id128[:Q, :Q])
    nc.scalar.activation(ucol[:].rearrange("p a b -> p (a b)"), ucol_ps[:],
                         AF.Copy)
    ucol_n = const.tile([128, NCH, Q], F32)
    nc.vector.tensor_scalar(ucol_n[:].rearrange("p a b -> p (a b)"),
                            ucol[:].rearrange("p a b -> p (a b)"),
                            -1.0, None, op0=OP.mult)

    # logits u-terms on PE early (ready at u_sb; emitted before anything
    # that would block the in-order PE queue)
    l_ps = ps_l.tile([128, M], F32, tag="l")
    l2_ps = ps_l2.tile([32, M], F32, tag="l2")
    for h in range(2):
        cols = slice(h * 512, (h + 1) * 512)
        nc.tensor.matmul(l_ps[:, cols], esel_coef[:], u_sb[:, cols],
                         start=True, stop=False, skip_group_check=True)
        nc.tensor.matmul(l2_ps[:, cols], esel2_coef[:], u_sb[:, cols],
                         start=True, stop=False, skip_group_check=True)

    # ---------------- ubc broadcasts ---------------------------------------
    ubc = big.tile([128, Q, M], F32)
    nc.gpsimd.partition_broadcast(ubc[:, 0, :], u_sb[0:1, :])
    for q in bcast_order():
        src = u_dram.ap()[q:q + 1, :].partition_broadcast(128)
        nc.sync.dma_start(ubc[:, q, :], src)

    # ---------------- main loop --------------------------------------------
    rcol = big.tile([128, NCH * Q], F32)
    nc.vector.memset(rcol[:], 0.0)   # M-queries' columns stay 0
    scr_a = big.tile([128, M], F32)
    scr_d = big.tile([128, M], F32)
    scr_m = [big.tile([128, M], F32R, tag=f"sm{i}", name=f"scr_m{i}")
             for i in range(3)]

    r_ps = ps_sc.tile([Q, M], F32, tag="sc")
    m_started = [not have_m, not have_m]  # per half: r_ps group started?
    mi = 0

    for kind, q, c, e in sched:
        if kind == "J":
            acc = rcol[:, Q * c + q:Q * c + q + 1]
            if e == "A":
                nc.scalar.activation(scr_a[:], ubc[:, q, :], AF.Relu,
                                     bias=ucol_n[:, c, q:q + 1], scale=1.0,
                                     accum_out=acc)
            else:
                nc.vector.scalar_tensor_tensor(scr_d[:], ubc[:, q, :],
                                               ucol[:, c, q:q + 1], zeros[:],
                                               op0=OP.subtract, op1=OP.max,
                                               accum_out=acc)
        else:
            scr = scr_m[mi % 3]
            mi += 1
            if os.environ.get("DKNN_MOP", "submin") == "sub":
                nc.vector.tensor_scalar(scr[:], ubc[:, q, :],
                                        ucol[:, c, q:q + 1], None,
                                        op0=OP.subtract)
            else:
                nc.vector.tensor_scalar(scr[:], ubc[:, q, :],
                                        ucol[:, c, q:q + 1], 0.0,
                                        op0=OP.subtract, op1=OP.min)
            for h in range(2):
                cols = slice(h * 512, (h + 1) * 512)
                nc.tensor.matmul(r_ps[:, cols], ohq[:, q, :], scr[:, cols],
                                 start=not m_started[h], stop=False,
                                 skip_group_check=True)
                m_started[h] = True

    # ---------------- epilogue ---------------------------------------------
    # Accumulate rcol^T into r_ps (PE transposes), copy PSUM->SBUF per
    # quarter, feed -2*acc into the logits, track the running row-max.
    r_sb = epil.tile([Q, M], F32)
    mxq1 = epil.tile([128, 4], F32)
    mxq2 = epil.tile([32, 4], F32)
    for qt in range(4):
        cols = slice(qt * 256, (qt + 1) * 256)
        for c in (2 * qt, 2 * qt + 1):
            nc.tensor.matmul(r_ps[:, c * 128:(c + 1) * 128],
                             rcol[:, c * Q:(c + 1) * Q], id128[:],
                             is_transpose=True, start=not have_m,
                             stop=(qt == 3 and c == 2 * qt + 1),
                             skip_group_check=True)
        nc.vector.tensor_copy(r_sb[:, cols], r_ps[:, cols])
        if r_dram is not None:
            nc.sync.dma_start(r_dram[:, cols], r_sb[:, cols])
        nc.tensor.matmul(l_ps[:, cols], esel_neg[:], r_sb[:, cols],
                         start=False, stop=(qt == 3), skip_group_check=True)
        nc.tensor.matmul(l2_ps[:, cols], esel2_neg[:], r_sb[:, cols],
                         start=False, stop=(qt == 3), skip_group_check=True)
        nc.vector.tensor_reduce(mxq1[:, qt:qt + 1], l_ps[:, cols],
                                axis=AX.X, op=OP.max)
        nc.vector.tensor_reduce(mxq2[:, qt:qt + 1], l2_ps[:, cols],
                                axis=AX.X, op=OP.max)

    # softmax rows: max, exp(+accum), 1/z folded into K-row-sum matmul
    mx2n = epil.tile([32, 1], F32)
    nc.vector.tensor_reduce(mx2n[:], mxq2[:], axis=AX.X, op=OP.max,
                            negate=True)
    mx1n = epil.tile([128, 1], F32)
    nc.vector.tensor_reduce(mx1n[:], mxq1[:], axis=AX.X, op=OP.max,
                            negate=True)

    p2 = epil.tile([32, M], F32R)
    z2 = epil.tile([32, 1], F32)
    nc.scalar.activation(p2[:], l2_ps[:], AF.Exp, bias=mx2n[:], scale=1.0,
                         accum_out=z2[:])
    p1 = epil.tile([128, M], F32R)
    z1 = epil.tile([128, 1], F32)
    nc.scalar.activation(p1[:], l_ps[:], AF.Exp, bias=mx1n[:], scale=1.0,
                         accum_out=z1[:])

    rz1 = epil.tile([128, 1], F32)
    nc.vector.reciprocal(rz1[:], z1[:])
    rz2 = epil.tile([32, 1], F32)
    nc.vector.reciprocal(rz2[:], z2[:])

    # sel[p, q] = rz[p] * [q(p) == q], fp32r
    def sel_mat(nparts, stride, rz, tag):
        vi = epil.tile([nparts, Q], I32, tag=f"svi{tag}")
        nc.gpsimd.iota(vi[:], pattern=[[-stride, Q]], base=0,
                       channel_multiplier=1)
        sh = stride.bit_length() - 1
        nc.vector.tensor_scalar(vi[:], vi[:], sh, None,
                                op0=OP.arith_shift_right)
        nc.vector.tensor_scalar(vi[:], vi[:], 0, None, op0=OP.is_equal)
        sf = epil.tile([nparts, Q], F32R, tag=f"ssf{tag}")
        nc.vector.tensor_copy(sf[:], vi[:])
        nc.vector.tensor_scalar(sf[:], sf[:], rz[:], None, op0=OP.mult)
        return sf

    sel2 = sel_mat(32, 2, rz2, "b")
    sel1 = sel_mat(128, 8, rz1, "a")

    out_ps = ps_sc.tile([Q, M], F32, tag="sc")
    for h in range(2):
        cols = slice(h * 512, (h + 1) * 512)
        nc.tensor.matmul(out_ps[:, cols], sel2[:], p2[:, cols],
                         start=True, stop=False)
        nc.tensor.matmul(out_ps[:, cols], sel1[:], p1[:, cols],
                         start=False, stop=True)

    out_sb = epil.tile([Q, M], F32)
    nc.scalar.activation(out_sb[:], out_ps[:], AF.Copy)
    nc.sync.dma_start(out_dram[:, :], out_sb[:])


_NC_CACHE = []


def kernel(query: np.ndarray, neighbors: np.ndarray) -> np.ndarray:
    query = np.ascontiguousarray(np.asarray(query, dtype=np.float32))
    neighbors = np.ascontiguousarray(np.asarray(neighbors, dtype=np.float32))
    assert query.shape == (QF, D) and neighbors.shape == (M, D)

    if not _NC_CACHE:
        _NC_CACHE.append(build_kernel())
    nc = _NC_CACHE[0]

    in_maps = [
        {"query": query[c * Q:(c + 1) * Q], "neighbors": neighbors}
        for c in range(NCORES)
    ]
    res = run_bass_kernel_spmd(nc, in_maps, core_ids=list(range(NCORES)))
    return np.concatenate([r["out"] for r in res.results], axis=0)


if __name__ == "__main__":
    rng = np.random.default_rng(0)
    q = rng.standard_normal((QF, D), dtype=np.float32)
    n = rng.standard_normal((M, D), dtype=np.float32)
    out = kernel(q, n)
    print("out", out.shape, out.dtype, out[0, :4])


# revision 28
# speedup vs baseline: 1.1980x; 1.1980x over previous
"""DKNN (retrieval_knn) Trainium2 Bass kernel — v5.

Full inputs: query [128, 256] f32, neighbors [1024, 256] f32.
Output: [128, 1024] f32 = sum of first K=10 NeuralSort P_hat rows.

Sharding: query batch split 16/core across 8 cores; neighbors replicated.

Math (per core, Q=16, M=1024, D=256, K=10, tau=1):
  u[q,m]   = 2*query[q]@n[m] - |n[m]|^2   (score + per-q const, cancels in
                                           softmax); then u -= max_m u[q,m]
  acc[q,j] = sum_m relu(u[q,m] - u[q,j])
  r = 2*acc + M*u - sum(u)
  logits[q,i,j] = u_j*(M-1-2i) - r_j
               == -(1+2i)*u_j - 2*acc_j + row_const   (M*u folds into the
                  coef exactly since M = 2^10; row consts cancel in softmax)
  out[q,j] = sum_i softmax_j(logits[q,i,:])

Numerics: acc partial sums are tiny exactly at the top-ranked j where the
softmax mass lives, so fp32 accumulation / fp32r PE-reduction noise lands
only on logits thousands below the row max. The -(1+2i) coefficient also
kills the ~1000x amplification of u's fp32 matmul noise (this was the
previous version's dominant error).

Engine split of the O(Q*M^2) work, per HW-measured op costs (gpsimd
supports no accumulating/2-tensor ops and its plain tensor_scalar ucode
is ~15us per [128,1024] op, so Pool only does the q0 broadcast; ACT
SBUF-source ops pay the 2.3x cayman errata):
  M-queries (most): 8 ops [p=m-chunk, free=j] on DVE:
      tensor_scalar min(u_j - u_m, 0) -> fp32r scratch (~594ns, 2x_2p);
      PE one-hot(-1) fp32r matmuls accumulate partial acc rows straight
      into r_ps PSUM (~480ns/op, PE is otherwise idle).
  J-queries (a few): 8 ops [p=j-chunk, free=m] on ACT (fused
      activation(Relu, bias=-u_j, accum_out) -> acc columns, ~2.1us) and
      a few on DVE (scalar_tensor_tensor + accum, ~1.2us); columns are
      PE-transposed (accumulating) into the same r_ps rows.

ubc (u[q,:] broadcast to 128 partitions) serves BOTH layouts (free dim
is m or j respectively). Engine SBUF APs must start at partition
0/32/64/96, so only q0 can use gpsimd partition_broadcast; the rest are
SP-queue DMA broadcast reads of a DRAM u copy (0-partition-stride AP),
delivered J-queries-first so ACT starts early.

Epilogue on PE: l[p,j] = coef'(p)*u[q(p),j] - 2*acc[q(p),j] built in
PSUM via one-hot replicate matmuls (u-terms early; -2*acc terms per
quarter as r_sb lands), then row-max + exp(+accum) on ACT, 1/z folded
into the final K-row-sum fp32r matmul.
"""

import os
from contextlib import ExitStack

import numpy as np

import concourse.bacc as bacc
import concourse.bass as bass
import concourse.tile as tile
import concourse.mybir as mybir
from concourse.bass_utils import run_bass_kernel_spmd

F32 = mybir.dt.float32
F32R = mybir.dt.float32r
I32 = mybir.dt.int32
AX = mybir.AxisListType
OP = mybir.AluOpType
AF = mybir.ActivationFunctionType

QF = 128          # full query batch
Q = 16            # queries per core
M = 1024          # neighbors
D = 256           # dim
K = 10            # top-k / NeuralSort rows
NCORES = 8
NCH = 8           # 128-row chunks per query (j-chunks or m-chunks)

# --- tuning knobs ---------------------------------------------------------
NJQ = int(os.environ.get("DKNN_NJQ", "4"))       # queries done J-side
# per-op engine cost estimates (ns) for the build-time list scheduler
COST_A = float(os.environ.get("DKNN_COST_A", "2110"))    # ACT J fused
COST_DJ = float(os.environ.get("DKNN_COST_DJ", "1200"))  # DVE J stt
COST_DM = float(os.environ.get("DKNN_COST_DM", "620"))   # DVE M pass
COST_PB = 1300.0          # pool partition_broadcast (q0)
COST_DB = 1650.0          # SP dma broadcast (queue-serial)


def j_queries():
    """J-side (ACT) queries: early ubc-delivery slots, interleaved with
    M-queries so DVE never starves."""
    return set(range(1, 2 * NJQ, 2))


def bcast_order():
    """SP DMA delivery order for ubc[1..15]: alternate J/M early."""
    jq = sorted(j_queries())
    mqs = [q for q in range(1, Q) if q not in jq]
    order = []
    i = j = 0
    while i < len(jq) or j < len(mqs):
        if i < len(jq):
            order.append(jq[i]); i += 1
        if j < len(mqs):
            order.append(mqs[j]); j += 1
    return order


def schedule_main_ops():
    """Greedy earliest-finish schedule.

    Returns ops as (kind, q, c, engine): kind 'J' (c = j-chunk) or 'M'
    (c = m-chunk). Engines: A (J-fused), D (M-pass or J-stt).
    Estimates guide balance only; correctness is schedule-independent.
    """
    jq = j_queries()
    ready = {0: COST_PB}
    t = 1650.0
    for q in bcast_order():
        t += COST_DB
        ready[q] = t

    jops = [(c, q) for c in range(NCH) for q in sorted(jq)]
    mops = [(mc, q) for q in range(Q) if q not in jq for mc in range(NCH)]
    avail = {"A": 0.0, "D": 0.0}
    sched = []
    while jops or mops:
        e = min("AD", key=lambda x: avail[x])
        if e == "A":
            if not jops:
                avail["A"] = float("inf")
                continue
            pool, cost, kind = jops, COST_A, "J"
        else:
            if mops:
                pool, cost, kind = mops, COST_DM, "M"
            else:
                pool, cost, kind = jops, COST_DJ, "J"
        op = min(pool, key=lambda cq: (max(ready[cq[1]], avail[e]), cq[0]))
        pool.remove(op)
        avail[e] = max(avail[e], ready[op[1]]) + cost
        sched.append((kind, op[1], op[0], e))
    return sched


def build_kernel():
    nc = bacc.Bacc(
        "TRN2",
        target_bir_lowering=False,
        debug=False,
        enable_asserts=False,
        num_devices=NCORES,
    )

    q_dram = nc.dram_tensor("query", [Q, D], F32, kind="ExternalInput")
    n_dram = nc.dram_tensor("neighbors", [M, D], F32, kind="ExternalInput")
    out_dram = nc.dram_tensor("out", [Q, M], F32, kind="ExternalOutput")
    dump = os.environ.get("DKNN_DEBUG_DUMP", "0") == "1"
    u_dram = nc.dram_tensor("u_scratch", [Q, M], F32,
                            kind="ExternalOutput" if dump else "Internal")
    r_dram = nc.dram_tensor("r_scratch", [Q, M], F32, kind="ExternalOutput") \
        if dump else None

    loop_n = int(os.environ.get("DKNN_LOOP", "1"))
    with tile.TileContext(nc) as tc:
        with ExitStack() as ctx:
            if loop_n > 1:
                with tc.For_i(0, loop_n, 1):
                    kernel_body(ctx, tc, q_dram, n_dram, out_dram, u_dram,
                                r_dram)
            else:
                kernel_body(ctx, tc, q_dram, n_dram, out_dram, u_dram,
                            r_dram)

    nc.compile()
    return nc


def kernel_body(ctx, tc, q_dram, n_dram, out_dram, u_dram, r_dram=None):
    nc = tc.nc

    const = ctx.enter_context(tc.tile_pool(name="const", bufs=1))
    big = ctx.enter_context(tc.tile_pool(name="big", bufs=1))
    epil = ctx.enter_context(tc.tile_pool(name="epil", bufs=1))
    ps_a = ctx.enter_context(tc.tile_pool(name="ps_a", bufs=2, space="PSUM"))
    ps_sc = ctx.enter_context(tc.tile_pool(name="ps_sc", bufs=1, space="PSUM"))
    ps_l = ctx.enter_context(tc.tile_pool(name="ps_l", bufs=1, space="PSUM"))
    ps_l2 = ctx.enter_context(tc.tile_pool(name="ps_l2", bufs=1, space="PSUM"))

    sched = schedule_main_ops()
    have_m = any(k == "M" for k, _, _, _ in sched)

    # ---------------- constants (no data deps) -----------------------------
    id128 = const.tile([128, 128], F32)
    idi = const.tile([128, 128], I32)
    nc.gpsimd.iota(idi[:], pattern=[[-1, 128]], base=0, channel_multiplier=1)
    nc.vector.tensor_scalar(idi[:], idi[:], 0, None, op0=OP.is_equal)
    nc.vector.tensor_copy(id128[:], idi[:])

    negones = const.tile([128, Q], F32)
    nc.vector.memset(negones[:], -1.0)
    zeros = const.tile([128, M], F32)
    nc.vector.memset(zeros[:], 0.0)

    # one-hot stack for M-side reduction: ohq[k, q, i] = -(i == q), fp32r
    ohi = const.tile([128, Q, Q], I32)
    nc.gpsimd.iota(ohi[:], pattern=[[-1, Q], [1, Q]], base=0,
                   channel_multiplier=0)
    nc.vector.tensor_scalar(ohi[:], ohi[:], 0, None, op0=OP.is_equal)
    ohq = const.tile([128, Q, Q], F32R)
    nc.vector.tensor_copy(ohq[:], ohi[:])
    nc.vector.tensor_scalar(ohq[:], ohq[:], -1.0, None, op0=OP.mult)

    # esel[k, p] one-hot replicators with coef'/-2 folded in
    def repsel(nparts, rep, scale_col=None, scale=None, tag=""):
        vi = epil.tile([Q, nparts], I32, tag=f"vi{tag}")
        nc.gpsimd.iota(vi[:], pattern=[[1, nparts]], base=0,
                       channel_multiplier=-rep)
        nc.vector.tensor_scalar(vi[:], vi[:], rep.bit_length() - 1, None,
                                op0=OP.arith_shift_right)
        nc.vector.tensor_scalar(vi[:], vi[:], 0, None, op0=OP.is_equal)
        sf = epil.tile([Q, nparts], F32, tag=f"sf{tag}")
        nc.vector.tensor_copy(sf[:], vi[:])
        if scale_col is not None:
            nc.vector.tensor_tensor(sf[:], sf[:], scale_col[:], op=OP.mult)
        if scale is not None:
            nc.vector.tensor_scalar(sf[:], sf[:], scale, None, op0=OP.mult)
        return sf

    # coef'(p) = -(1 + 2*i(p)): i = p%8 on 128 parts, 8 + p%2 on 32 parts
    def coef_row(nparts, mask, base_val, tag):
        it = epil.tile([Q, nparts], I32, tag=f"it{tag}")
        nc.gpsimd.iota(it[:], pattern=[[1, nparts]], base=0,
                       channel_multiplier=0)
        nc.vector.tensor_scalar(it[:], it[:], mask, None, op0=OP.bitwise_and)
        ft = epil.tile([Q, nparts], F32, tag=f"ft{tag}")
        nc.vector.tensor_copy(ft[:], it[:])
        nc.vector.tensor_scalar(ft[:], ft[:], -2.0, float(base_val),
                                op0=OP.mult, op1=OP.add)
        return ft

    coefr1 = coef_row(128, 7, -1, "a")
    coefr2 = coef_row(32, 1, -17, "b")
    esel_coef = repsel(128, 8, scale_col=coefr1, tag="ec")
    esel2_coef = repsel(32, 2, scale_col=coefr2, tag="ec2")
    esel_neg = repsel(128, 8, scale=-2.0, tag="en")
    esel2_neg = repsel(32, 2, scale=-2.0, tag="en2")

    # ---------------- load inputs ------------------------------------------
    n_sb = big.tile([128, NCH, D], F32)
    n_re = n_dram.ap().rearrange("(mc p) d -> p mc d", p=128)
    for mc in range(NCH):
        eng = (nc.sync, nc.scalar)[mc % 2]
        eng.dma_start(n_sb[:, mc, :], n_re[:, mc, :])

    q_sb = const.tile([Q, D], F32)
    nc.sync.dma_start(q_sb[:], q_dram[:, :])

    # nT[p, dc, m] = neighbors[m, 128*dc + p] via 16 PE transposes
    nT = big.tile([128, 2, M], F32)
    for dc in range(2):
        for hh in range(2):
            psT = ps_a.tile([128, 512], F32, tag="a")
            for mi in range(4):
                mc = hh * 4 + mi
                nc.tensor.transpose(
                    psT[:, mi * 128:(mi + 1) * 128],
                    n_sb[:, mc, dc * 128:(dc + 1) * 128],
                    id128[:],
                )
            eng = nc.scalar if (dc + hh) % 2 == 0 else nc.vector
            if eng is nc.scalar:
                eng.activation(nT[:, dc, hh * 512:(hh + 1) * 512], psT[:],
                               AF.Copy)
            else:
                eng.tensor_copy(nT[:, dc, hh * 512:(hh + 1) * 512], psT[:])

    # q2T[p, dc, q] = 2 * query[q, 128*dc + p]
    q2T = const.tile([128, 2, Q], F32)
    psQ = ps_a.tile([128, 2 * Q], F32, tag="a")
    for dc in range(2):
        nc.tensor.transpose(
            psQ[:, dc * Q:(dc + 1) * Q],
            q_sb[:, dc * 128:(dc + 1) * 128],
            id128[:Q, :Q],
        )
    nc.vector.tensor_scalar(
        q2T[:].rearrange("p a b -> p (a b)"), psQ[:], 2.0, None, op0=OP.mult)

    # nsq = nT*nT (the constant D-offset of u is m-independent and cancels
    # via max-centering; PSUM partial noise is tolerable now that the
    # logit coefficient is -(1+2i) rather than ~1000)
    nsq = big.tile([128, 2, M], F32)
    nc.vector.tensor_tensor(nsq[:], nT[:], nT[:], op=OP.mult)

    # u[q, m] = sum_d 2*q[q,d]*n[m,d] - sum_d n[m,d]^2
    scores_ps = ps_sc.tile([Q, M], F32, tag="sc")
    for h in range(2):  # 512-column halves (fp32 moving-operand limit)
        cols = slice(h * 512, (h + 1) * 512)
        nc.tensor.matmul(scores_ps[:, cols], q2T[:, 0, :], nT[:, 0, cols],
                         start=True, stop=False)
        nc.tensor.matmul(scores_ps[:, cols], q2T[:, 1, :], nT[:, 1, cols],
                         start=False, stop=False)
        nc.tensor.matmul(scores_ps[:, cols], negones[:], nsq[:, 0, cols],
                         start=False, stop=False)
        nc.tensor.matmul(scores_ps[:, cols], negones[:], nsq[:, 1, cols],
                         start=False, stop=True)

    # u to SBUF, max-centered per query (folded into the PSUM->SBUF copy)
    umax = const.tile([Q, 1], F32)
    umaxn = const.tile([Q, 1], F32)
    nc.vector.tensor_reduce(umax[:], scores_ps[:], axis=AX.X, op=OP.max)
    nc.vector.tensor_scalar(umaxn[:], umax[:], -1.0, None, op0=OP.mult)
    u_sb = const.tile([Q, M], F32)
    nc.scalar.activation(u_sb[:], scores_ps[:], AF.Identity, bias=umaxn[:],
                         scale=1.0)
    nc.sync.dma_start(u_dram[:, :], u_sb[:])

    # ucol[p, c, q] = u[q, 128*c + p], plus negated copy (ACT relu bias)
    ucol = const.tile([128, NCH, Q], F32)
    ucol_ps = ps_a.tile([128, NCH * Q], F32, tag="a")
    for c in range(NCH):
        nc.tensor.transpose(ucol_ps[:, c * Q:(c + 1) * Q],
                            u_sb[:, c * 128:(c + 1) * 128], id128[:Q, :Q])
    nc.scalar.activation(ucol[:].rearrange("p a b -> p (a b)"), ucol_ps[:],
                         AF.Copy)
    ucol_n = const.tile([128, NCH, Q], F32)
    nc.vector.tensor_scalar(ucol_n[:].rearrange("p a b -> p (a b)"),
                            ucol[:].rearrange("p a b -> p (a b)"),
                            -1.0, None, op0=OP.mult)

    # logits u-terms on PE early (ready at u_sb; emitted before anything
    # that would block the in-order PE queue)
    l_ps = ps_l.tile([128, M], F32, tag="l")
    l2_ps = ps_l2.tile([32, M], F32, tag="l2")
    for h in range(2):
        cols = slice(h * 512, (h + 1) * 512)
        nc.tensor.matmul(l_ps[:, cols], esel_coef[:], u_sb[:, cols],
                         start=True, stop=False, skip_group_check=True)
        nc.tensor.matmul(l2_ps[:, cols], esel2_coef[:], u_sb[:, cols],
                         start=True, stop=False, skip_group_check=True)

    # ---------------- ubc broadcasts ---------------------------------------
    ubc = big.tile([128, Q, M], F32)
    nc.gpsimd.partition_broadcast(ubc[:, 0, :], u_sb[0:1, :])
    UBC0 = os.environ.get("DKNN_UBC0", "0") == "1"  # timing expt: no DMAs
    if not UBC0:
        for q in bcast_order():
            src = u_dram.ap()[q:q + 1, :].partition_broadcast(128)
            nc.sync.dma_start(ubc[:, q, :], src)

    # ---------------- main loop --------------------------------------------
    rcol = big.tile([128, NCH * Q], F32)
    nc.vector.memset(rcol[:], 0.0)   # M-queries' columns stay 0
    scr_a = big.tile([128, M], F32)
    scr_d = big.tile([128, M], F32)
    scr_m = [big.tile([128, M], F32R, tag=f"sm{i}", name=f"scr_m{i}")
             for i in range(3)]

    r_ps = ps_sc.tile([Q, M], F32, tag="sc")
    m_started = [not have_m, not have_m]  # per half: r_ps group started?
    mi = 0

    for kind, q0_, c, e in sched:
        q = 0 if UBC0 else q0_
        if kind == "J":
            acc = rcol[:, Q * c + q0_:Q * c + q0_ + 1]
            if e == "A":
                nc.scalar.activation(scr_a[:], ubc[:, q, :], AF.Relu,
                                     bias=ucol_n[:, c, q0_:q0_ + 1],
                                     scale=1.0, accum_out=acc)
            else:
                nc.vector.scalar_tensor_tensor(scr_d[:], ubc[:, q, :],
                                               ucol[:, c, q0_:q0_ + 1],
                                               zeros[:], op0=OP.subtract,
                                               op1=OP.max, accum_out=acc)
        else:
            scr = scr_m[mi % 3]
            mi += 1
            if os.environ.get("DKNN_MOP", "submin") == "sub":
                nc.vector.tensor_scalar(scr[:], ubc[:, q, :],
                                        ucol[:, c, q0_:q0_ + 1], None,
                                        op0=OP.subtract)
            else:
                nc.vector.tensor_scalar(scr[:], ubc[:, q, :],
                                        ucol[:, c, q0_:q0_ + 1], 0.0,
                                        op0=OP.subtract, op1=OP.min)
            for h in range(2):
                cols = slice(h * 512, (h + 1) * 512)
                nc.tensor.matmul(r_ps[:, cols], ohq[:, q0_, :],
                                 scr[:, cols],
                                 start=not m_started[h], stop=False,
                                 skip_group_check=True)
                m_started[h] = True

    # ---------------- epilogue ---------------------------------------------
    # Accumulate rcol^T into r_ps (PE transposes), copy PSUM->SBUF per
    # quarter, feed -2*acc into the logits, track the running row-max.
    r_sb = epil.tile([Q, M], F32)
    mxq1 = epil.tile([128, 4], F32)
    mxq2 = epil.tile([32, 4], F32)
    for qt in range(4):
        cols = slice(qt * 256, (qt + 1) * 256)
        for c in (2 * qt, 2 * qt + 1):
            nc.tensor.matmul(r_ps[:, c * 128:(c + 1) * 128],
                             rcol[:, c * Q:(c + 1) * Q], id128[:],
                             is_transpose=True, start=not have_m,
                             stop=(qt == 3 and c == 2 * qt + 1),
                             skip_group_check=True)
        nc.vector.tensor_copy(r_sb[:, cols], r_ps[:, cols])
        if r_dram is not None:
            nc.sync.dma_start(r_dram[:, cols], r_sb[:, cols])
        nc.tensor.matmul(l_ps[:, cols], esel_neg[:], r_sb[:, cols],
                         start=False, stop=(qt == 3), skip_group_check=True)
        nc.tensor.matmul(l2_ps[:, cols], esel2_neg[:], r_sb[:, cols],
                         start=False, stop=(qt == 3), skip_group_check=True)
        nc.vector.tensor_reduce(mxq1[:, qt:qt + 1], l_ps[:, cols],
                                axis=AX.X, op=OP.max)
        nc.vector.tensor_reduce(mxq2[:, qt:qt + 1], l2_ps[:, cols],
                                axis=AX.X, op=OP.max)

    # softmax rows: max, exp(+accum), 1/z folded into K-row-sum matmul
    mx2n = epil.tile([32, 1], F32)
    nc.vector.tensor_reduce(mx2n[:], mxq2[:], axis=AX.X, op=OP.max,
                            negate=True)
    mx1n = epil.tile([128, 1], F32)
    nc.vector.tensor_reduce(mx1n[:], mxq1[:], axis=AX.X, op=OP.max,
                            negate=True)

    p2 = epil.tile([32, M], F32R)
    z2 = epil.tile([32, 1], F32)
    nc.scalar.activation(p2[:], l2_ps[:], AF.Exp, bias=mx2n[:], scale=1.0,
                         accum_out=z2[:])
    p1 = epil.tile([128, M], F32R)
    z1 = epil.tile([128, 1], F32)
    nc.scalar.activation(p1[:], l_ps[:], AF.Exp, bias=mx1n[:], scale=1.0,
                         accum_out=z1[:])

    rz1 = epil.tile([128, 1], F32)
    nc.vector.reciprocal(rz1[:], z1[:])
    rz2 = epil.tile([32, 1], F32)
    nc.vector.reciprocal(rz2[:], z2[:])

    # sel[p, q] = rz[p] * [q(p) == q], fp32r
    def sel_mat(nparts, stride, rz, tag):
        vi = epil.tile([nparts, Q], I32, tag=f"svi{tag}")
        nc.gpsimd.iota(vi[:], pattern=[[-stride, Q]], base=0,
                       channel_multiplier=1)
        sh = stride.bit_length() - 1
        nc.vector.tensor_scalar(vi[:], vi[:], sh, None,
                                op0=OP.arith_shift_right)
        nc.vector.tensor_scalar(vi[:], vi[:], 0, None, op0=OP.is_equal)
        sf = epil.tile([nparts, Q], F32R, tag=f"ssf{tag}")
        nc.vector.tensor_copy(sf[:], vi[:])
        nc.vector.tensor_scalar(sf[:], sf[:], rz[:], None, op0=OP.mult)
        return sf

    sel2 = sel_mat(32, 2, rz2, "b")
    sel1 = sel_mat(128, 8, rz1, "a")

    out_ps = ps_sc.tile([Q, M], F32, tag="sc")
    for h in range(2):
        cols = slice(h * 512, (h + 1) * 512)
        nc.tensor.matmul(out_ps[:, cols], sel2[:], p2[:, cols],
                         start=True, stop=False)
        nc.tensor.matmul(out_ps[:, cols], sel1[:], p1[:, cols],
                         start=False, stop=True)

    out_sb = epil.tile([Q, M], F32)
    nc.scalar.activation(out_sb[:], out_ps[:], AF.Copy)
    nc.sync.dma_start(out_dram[:, :], out_sb[:])


_NC_CACHE = []


def kernel(query: np.ndarray, neighbors: np.ndarray) -> np.ndarray:
    query = np.ascontiguousarray(np.asarray(query, dtype=np.float32))
    neighbors = np.ascontiguousarray(np.asarray(neighbors, dtype=np.float32))
    assert query.shape == (QF, D) and neighbors.shape == (M, D)

    if not _NC_CACHE:
        _NC_CACHE.append(build_kernel())
    nc = _NC_CACHE[0]

    in_maps = [
        {"query": query[c * Q:(c + 1) * Q], "neighbors": neighbors}
        for c in range(NCORES)
    ]
    res = run_bass_kernel_spmd(nc, in_maps, core_ids=list(range(NCORES)))
    return np.concatenate([r["out"] for r in res.results], axis=0)


if __name__ == "__main__":
    rng = np.random.default_rng(0)
    q = rng.standard_normal((QF, D), dtype=np.float32)
    n = rng.standard_normal((M, D), dtype=np.float32)
    out = kernel(q, n)
    print("out", out.shape, out.dtype, out[0, :4])
